# revision 49
# baseline (speedup 1.0000x reference)
"""Causal single-head attention (B=16, S=2048, D=1024, HD=64) on 8 TRN2 cores.

Data-parallel: 2 batches per core, all matmuls bf16 (fp32 PSUM accumulate).

Host side: x is cast to bf16 and pre-transposed to [BPC, D, S] so the
contraction dim lands on SBUF partitions via plain DMA -- no PE transposes.
Weights are packed as stationary pairs [Wk|Wq] and [Wq|Wv] in bf16.  The
kernel returns UNNORMALIZED O with a 65th denominator column; the softmax
divide runs on host.

Device side, per seq-tile pair (both batches at the same qtile):
  - projections: stationary [Wk|Wq] / [Wq|Wv] over X^T d-blocks into one
    2-bank PSUM tile; PSUM->SBUF copies on DVE add the biases and narrow
    to bf16 (K^T kept for S, Q^T|V^T kept as the qv tile),
  - V natural (plus a ones column for the denominators) via 4 small PE
    transposes of V^T,
  - attention interleaves the two batches: S^T = K @ Q^T for batch A and
    B land in the two banks of one PSUM tile (sharing one bank on the
    narrow diagonal blocks), one ScalarE exp covers both (scale
    1/sqrt(64) folded in), causal diag masked on DVE,
  - O is accumulated in NATURAL layout: the exp'd score chunk [128k,128q]
    is the STATIONARY operand and V-natural [128k, 65] the moving one;
    all 4 query-chunk accumulation groups share one PSUM bank (first
    matmul start=True pends-zero the bank, the rest overwrite-then-
    accumulate), the 65th column accumulating the denominators,
  - the o bank DMAs straight from PSUM to DRAM.

All xt DMAs are issued up front across the sync/gpsimd queues; later
units' projection work is pulled between attention blocks to keep the PE
dense, matmul heaters warm the HAM clock during the initial DMA fill and
through the exp-paced tail.
"""
import numpy as np
import ml_dtypes

import concourse.bacc as bacc
import concourse.mybir as mybir
import concourse.tile as tile
from concourse import bass_utils

B, S, D, HD = 16, 2048, 1024, 64
N_CORES = 8
BPC = B // N_CORES          # batches per core
ST = 512                    # seq tile (qtile) size
NST = S // ST               # 4 seq tiles per batch
NDB = D // 128              # 8 d-blocks
NKB = S // 128              # 16 kseq blocks per batch

f32 = mybir.dt.float32
bf16 = mybir.dt.bfloat16
BF = ml_dtypes.bfloat16

# bf16 consts layout (cols): wkq | wqv | ident | mask | ones
C_WKQ, C_WQV, C_ID, C_MASK, C_ONES = 0, 1024, 2048, 2176, 2304
C_TOT = 2320
N_HEAT = 16
PULLS = [4, 3, 2, 0]        # filler steps pulled per attention block, by pair

_cache = {}


def _build():
    nc = bacc.Bacc("TRN2", target_bir_lowering=False, debug=False,
                   num_devices=N_CORES)

    xt_d = nc.dram_tensor("xt", [BPC, D, S], bf16, kind="ExternalInput")
    cb_d = nc.dram_tensor("cb", [128, C_TOT], bf16, kind="ExternalInput")
    cf_d = nc.dram_tensor("cf", [128, 2], f32, kind="ExternalInput")
    # device-friendly layout: per partition 4*65 contiguous rows (a [S, 65]
    # row-major layout would DMA in 260B packets); bf16 halves the final
    # copy+DMA; host undoes layout, upcasts, and normalizes
    out = nc.dram_tensor("out", [BPC, NST, 128, 4, 65], bf16,
                         kind="ExternalOutput")

    with tile.TileContext(nc) as tc:
        with (
            tc.tile_pool(name="consts", bufs=1) as consts,
            tc.tile_pool(name="xtp", bufs=8) as xtp,
            tc.tile_pool(name="per", bufs=1) as per,
            tc.tile_pool(name="work", bufs=4) as work,
            tc.tile_pool(name="pp", bufs=1, space="PSUM") as pp,
            tc.tile_pool(name="ps_s", bufs=2, space="PSUM") as ps_s,
            tc.tile_pool(name="ps_o", bufs=3, space="PSUM") as ps_o,
        ):
            cb = consts.tile([128, C_TOT], bf16, name="cb")
            cf = consts.tile([128, 2], f32, name="cf")
            heat = consts.tile([128, 128], bf16, name="heat")
            nc.vector.memset(heat, 0.0)
            # weights on the scalar queue (idle until the first exp);
            # sync + gpsimd queues carry every xt tile up front
            nc.scalar.dma_start(out=cb[:, 0:C_ID], in_=cb_d.ap()[:, 0:C_ID])
            nc.scalar.dma_start(out=cb[:, C_ID:], in_=cb_d.ap()[:, C_ID:])
            nc.scalar.dma_start(out=cf, in_=cf_d.ap())

            units = [(b, st) for st in range(NST) for b in range(BPC)]
            xt_sbs = []
            qeng = [nc.sync, nc.gpsimd]
            for i, (b, st) in enumerate(units):
                xt_sb = xtp.tile([128, NDB, ST], bf16, tag="xt",
                                 bufs=8, name=f"xt_sb{i}")
                src = xt_d.ap()[b, :, ST * st:ST * (st + 1)]
                src = src.rearrange("(db p) s -> p db s", p=128)
                nds = 4 if i == 0 else (2 if i == 1 else 1)
                step = NDB // nds
                for k in range(nds):
                    qeng[(2 * i + k) % 2].dma_start(
                        out=xt_sb[:, k * step:(k + 1) * step, :],
                        in_=src[:, k * step:(k + 1) * step, :])
                xt_sbs.append(xt_sb)

            wkq = cb[:, C_WKQ:C_WKQ + 1024].rearrange("p (db m) -> p db m",
                                                      db=NDB)
            wqv = cb[:, C_WQV:C_WQV + 1024].rearrange("p (db m) -> p db m",
                                                      db=NDB)
            ident = cb[:, C_ID:C_ID + 128]
            mask = cb[:, C_MASK:C_MASK + 128]
            ones_c = cb[:, C_ONES:C_ONES + NKB]

            # matmul heater: keeps HAM busy through the initial DMA fill
            heat_ps = ps_o.tile([128, 4, 128], f32, tag="o", name="heat_ps")
            for _ in range(N_HEAT):
                nc.tensor.matmul(heat_ps[:, 0, :], heat, heat,
                                 start=True, stop=True)

            kq_sbs, vn_sbs = [], []
            for b in range(BPC):
                kq_sb = per.tile([128, S], bf16, name=f"kq_sb{b}")
                vn_sb = per.tile([128, NKB, 65], bf16, name=f"vn_sb{b}")
                # gpsimd: emitted after its xt dma_starts, so the queue
                # issues all loads before blocking on the consts DMA
                nc.gpsimd.tensor_copy(
                    out=vn_sb[:, :, 64:65],
                    in_=ones_c.rearrange("p (t o) -> p t o", o=1))
                kq_sbs.append(kq_sb)
                vn_sbs.append(vn_sb)

            def filler_gen(i, b, st):
                """Projections and V-natural prep for unit (b, st).  The
                two passes run through a single-bank PSUM ring (frees a
                bank for ps_o triple-buffering)."""
                kq_sb, vn_sb = kq_sbs[b], vn_sbs[b]
                xt_sb = xt_sbs[i]
                p1 = pp.tile([128, 512], f32, tag="p12", name="p1")
                for db in range(NDB):
                    nc.tensor.matmul(p1, wkq[:, db, :], xt_sb[:, db, :],
                                     start=(db == 0), stop=(db == NDB - 1))
                    if db % 2 == 1:
                        yield None
                # PSUM->SBUF with bias add, bf16 out
                nc.vector.tensor_scalar_add(
                    out=kq_sb[0:64, ST * st:ST * (st + 1)],
                    in0=p1[0:64, :], scalar1=cf[0:64, 0:1])
                yield None
                p2 = pp.tile([128, 512], f32, tag="p12", name="p2")
                for db in range(NDB):
                    nc.tensor.matmul(p2, wqv[:, db, :], xt_sb[:, db, :],
                                     start=(db == 0), stop=(db == NDB - 1))
                    if db % 2 == 1:
                        yield None
                qv_sb = work.tile([128, ST], bf16, tag="qv", bufs=4,
                                  name="qv_sb")
                nc.vector.tensor_scalar_add(out=qv_sb, in0=p2,
                                            scalar1=cf[:, 1:2])
                yield None
                vn_ps = pp.tile([128, 4, 64], bf16, tag="p12", name="vn_ps")
                for c in range(4):
                    nc.tensor.transpose(
                        vn_ps[:, c, :],
                        qv_sb[64:128, 128 * c:128 * (c + 1)],
                        ident[64:128, 64:128])
                nc.vector.tensor_copy(
                    out=vn_sb[:, 4 * st:4 * st + 4, 0:64], in_=vn_ps)
                yield qv_sb

            def fill_chain():
                for i, (b, st) in enumerate(units):
                    for r in filler_gen(i, b, st):
                        if r is not None:
                            yield ("unit", i, b, st, r)
                        else:
                            yield ("step", i)

            chain = fill_chain()
            qv_units = {}
            exhausted = [False]

            def pull_until_unit(i):
                for kind, *rest in chain:
                    if kind == "unit":
                        qv_units[rest[0]] = rest[3]
                        if rest[0] >= i:
                            return
                exhausted[0] = True

            def pull_steps(n):
                got = 0
                while got < n:
                    try:
                        kind, *rest = next(chain)
                    except StopIteration:
                        exhausted[0] = True
                        return
                    if kind == "unit":
                        qv_units[rest[0]] = rest[3]
                    else:
                        got += 1

            def emit_o(st, kb, bbase, e_sb, o_ts, last):
                j0 = max(kb - 4 * st, 0)
                for b in range(BPC):
                    base = bbase * b
                    for qc in range(j0, 4):
                        nc.tensor.matmul(
                            o_ts[b][:, qc, 0:65],
                            e_sb[:, base + 128 * (qc - j0):
                                 base + 128 * (qc - j0) + 128],
                            vn_sbs[b][:, kb, :],
                            start=(kb == 0 and qc == j0),
                            stop=(last and qc == 3))

            EXP = mybir.ActivationFunctionType.Exp
            scale = float(HD) ** -0.5
            for st in range(NST):
                iB = 2 * st + 1
                if iB not in qv_units:
                    pull_until_unit(iB)
                qv_A = qv_units.pop(iB - 1)
                qv_B = qv_units.pop(iB)
                o_ts = [ps_o.tile([128, 4, 128], f32, tag="o",
                                  name=f"o_t{b}") for b in range(BPC)]
                n_att = 4 * st + 4
                pend = None
                for kb in range(n_att):
                    j = kb - 4 * st
                    w = ST if j < 0 else ST - 128 * j
                    qoff = 0 if j < 0 else 128 * j
                    # batch B's bank offset: diag j>=2 packs into bank 0
                    bbase = 512 if w > 256 else w
                    s_ps = ps_s.tile([128, 1024], f32, tag="s", name="s_ps")
                    nc.tensor.matmul(
                        s_ps[:, 0:w],
                        kq_sbs[0][0:64, 128 * kb:128 * kb + 128],
                        qv_A[0:64, qoff:qoff + w],
                        start=True, stop=(bbase == 512))
                    nc.tensor.matmul(
                        s_ps[:, bbase:bbase + w],
                        kq_sbs[1][0:64, 128 * kb:128 * kb + 128],
                        qv_B[0:64, qoff:qoff + w],
                        start=(bbase == 512), stop=True)
                    e_sb = work.tile([128, 1024], bf16, tag="e", bufs=4,
                                     name="e_sb")
                    if w == ST or bbase != 512:
                        nc.scalar.activation(out=e_sb[:, 0:bbase + w],
                                             in_=s_ps[:, 0:bbase + w],
                                             func=EXP, scale=scale)
                    else:
                        nc.scalar.activation(out=e_sb[:, 0:w],
                                             in_=s_ps[:, 0:w],
                                             func=EXP, scale=scale)
                        nc.scalar.activation(out=e_sb[:, 512:512 + w],
                                             in_=s_ps[:, 512:512 + w],
                                             func=EXP, scale=scale)
                    if j >= 0:
                        nc.vector.tensor_mul(out=e_sb[:, 0:128],
                                             in0=e_sb[:, 0:128], in1=mask)
                        nc.vector.tensor_mul(
                            out=e_sb[:, bbase:bbase + 128],
                            in0=e_sb[:, bbase:bbase + 128], in1=mask)
                    if pend is not None:
                        emit_o(st, *pend, o_ts, last=False)
                    pend = (kb, bbase, e_sb)
                    pull_steps(PULLS[st])
                    if exhausted[0]:
                        # tail heater: keep the PE dense (HAM warm)
                        # through the exp-paced last blocks; runs in the
                        # idle projection-PSUM ring
                        heat_t = pp.tile([128, 512], f32, tag="p12",
                                         name="heat_t")
                        for _ in range(2):
                            nc.tensor.matmul(heat_t[:, 0:128], heat, heat,
                                             start=True, stop=True)
                # final block: per batch, finish O then immediately drain
                # that batch's o bank (copy + DMA) so the next pair's
                # start=True never waits on it
                kb_l, bbase_l, e_l = pend
                j0 = max(kb_l - 4 * st, 0)
                for b in range(BPC):
                    base = bbase_l * b
                    for qc in range(j0, 4):
                        nc.tensor.matmul(
                            o_ts[b][:, qc, 0:65],
                            e_l[:, base + 128 * (qc - j0):
                                base + 128 * (qc - j0) + 128],
                            vn_sbs[b][:, kb_l, :],
                            start=False, stop=(qc == 3))
                    o_sb = work.tile([128, 4, 65], bf16, tag="osb", bufs=4,
                                     name="o_sb")
                    nc.vector.tensor_copy(out=o_sb, in_=o_ts[b][:, :, 0:65])
                    nc.sync.dma_start(out=out.ap()[b, st], in_=o_sb)

    nc.compile()
    return nc


def _pack_consts(Wq, bq, Wk, bk, Wv, bv):
    def packed_pair(wa, wb):
        pa = wa.reshape(NDB, 128, HD).transpose(1, 0, 2)
        pb = wb.reshape(NDB, 128, HD).transpose(1, 0, 2)
        return np.concatenate([pa, pb], axis=2).reshape(128, NDB * 128)

    cb = np.zeros((128, C_TOT), dtype=np.float32)
    cb[:, C_WKQ:C_WKQ + 1024] = packed_pair(Wk, Wq)
    cb[:, C_WQV:C_WQV + 1024] = packed_pair(Wq, Wv)
    cb[:, C_ID:C_ID + 128] = np.eye(128, dtype=np.float32)
    cb[:, C_MASK:C_MASK + 128] = (
        np.arange(128)[None, :] >= np.arange(128)[:, None])
    cb[:, C_ONES:] = 1.0
    cf = np.zeros((128, 2), dtype=np.float32)
    cf[:, 0] = np.concatenate([bk, bq])
    cf[:, 1] = np.concatenate([bq, bv])
    return np.ascontiguousarray(cb.astype(BF)), np.ascontiguousarray(cf)


def kernel(x, Wq, bq, Wk, bk, Wv, bv):
    if "nc" not in _cache:
        _cache["nc"] = _build()
    nc = _cache["nc"]

    x = np.asarray(x, dtype=np.float32).reshape(N_CORES, BPC, S, D)
    xt = np.ascontiguousarray(x.astype(BF).transpose(0, 1, 3, 2))
    cb, cf = _pack_consts(np.asarray(Wq, np.float32),
                          np.asarray(bq, np.float32),
                          np.asarray(Wk, np.float32),
                          np.asarray(bk, np.float32),
                          np.asarray(Wv, np.float32),
                          np.asarray(bv, np.float32))

    in_maps = []
    for c in range(N_CORES):
        in_maps.append({"xt": xt[c], "cb": cb, "cf": cf})

    res = bass_utils.run_bass_kernel_spmd(nc, in_maps,
                                          core_ids=list(range(N_CORES)),
                                          **_cache.get("run_kwargs", {}))
    _cache["last_result"] = res
    o = np.concatenate([res.results[c]["out"] for c in range(N_CORES)],
                       axis=0).astype(np.float32)   # [B, NST, 128, 4, 65]
    o = o.transpose(0, 1, 3, 2, 4).reshape(B, S, 65)  # seq = st*512+t*128+p
    return np.ascontiguousarray(o[:, :, 0:64] / o[:, :, 64:65])


# revision 53
# speedup vs baseline: 1.1161x; 1.1161x over previous
"""Causal single-head attention (B=16, S=2048, D=1024, HD=64) on 8 TRN2 cores.

Data-parallel: 2 batches per core, all matmuls bf16 (fp32 PSUM accumulate).

Host side: x is cast to bf16 and pre-transposed to [BPC, D, S] so the
contraction dim lands on SBUF partitions via plain DMA -- no PE transposes.
Weights are packed as stationary pairs [Wk|Wq] and [Wq|Wv] in bf16.  The
kernel returns UNNORMALIZED O with a 65th denominator column; the softmax
divide runs on host.

Device side, per seq-tile pair (both batches at the same qtile):
  - projections: stationary [Wk|Wq] / [Wq|Wv] over X^T d-blocks into one
    2-bank PSUM tile; PSUM->SBUF copies on DVE add the biases and narrow
    to bf16 (K^T kept for S, Q^T|V^T kept as the qv tile),
  - V natural (plus a ones column for the denominators) via 4 small PE
    transposes of V^T,
  - attention interleaves the two batches: S^T = K @ Q^T for batch A and
    B land in the two banks of one PSUM tile (sharing one bank on the
    narrow diagonal blocks), one ScalarE exp covers both (scale
    1/sqrt(64) folded in), causal diag masked on DVE,
  - O is accumulated in NATURAL layout: the exp'd score chunk [128k,128q]
    is the STATIONARY operand and V-natural [128k, 65] the moving one;
    all 4 query-chunk accumulation groups share one PSUM bank (first
    matmul start=True pends-zero the bank, the rest overwrite-then-
    accumulate), the 65th column accumulating the denominators,
  - the o bank DMAs straight from PSUM to DRAM.

All xt DMAs are issued up front across the sync/gpsimd queues; later
units' projection work is pulled between attention blocks to keep the PE
dense, matmul heaters warm the HAM clock during the initial DMA fill and
through the exp-paced tail.
"""
import numpy as np
import ml_dtypes

import concourse.bacc as bacc
import concourse.mybir as mybir
import concourse.tile as tile
from concourse import bass_utils

B, S, D, HD = 16, 2048, 1024, 64
N_CORES = 8
BPC = B // N_CORES          # batches per core
ST = 512                    # seq tile (qtile) size
NST = S // ST               # 4 seq tiles per batch
NDB = D // 128              # 8 d-blocks
NKB = S // 128              # 16 kseq blocks per batch

f32 = mybir.dt.float32
bf16 = mybir.dt.bfloat16
BF = ml_dtypes.bfloat16

# bf16 consts layout (cols): wkq | wqv | ident | mask | ones
C_WKQ, C_WQV, C_ID, C_MASK, C_ONES = 0, 1024, 2048, 2176, 2304
C_TOT = 2320
N_HEAT = 16
PULLS = [4, 3, 2, 0]        # filler steps pulled per attention block, by pair

_cache = {}


def _build():
    nc = bacc.Bacc("TRN2", target_bir_lowering=False, debug=False,
                   num_devices=N_CORES)

    xt_d = nc.dram_tensor("xt", [BPC, D, S], bf16, kind="ExternalInput")
    cb_d = nc.dram_tensor("cb", [128, C_TOT], bf16, kind="ExternalInput")
    cf_d = nc.dram_tensor("cf", [128, 2], f32, kind="ExternalInput")
    # device-friendly layout: per partition 4*65*4B=1040B contiguous rows
    # (a [S, 65] row-major layout would DMA in 260B packets); host undoes it
    out = nc.dram_tensor("out", [BPC, NST, 128, 4, 65], f32,
                         kind="ExternalOutput")

    with tile.TileContext(nc) as tc:
        with (
            tc.tile_pool(name="consts", bufs=1) as consts,
            tc.tile_pool(name="xtp", bufs=8) as xtp,
            tc.tile_pool(name="per", bufs=1) as per,
            tc.tile_pool(name="work", bufs=4) as work,
            tc.tile_pool(name="pp", bufs=1, space="PSUM") as pp,
            tc.tile_pool(name="ps_s", bufs=2, space="PSUM") as ps_s,
            tc.tile_pool(name="ps_o", bufs=3, space="PSUM") as ps_o,
        ):
            cb = consts.tile([128, C_TOT], bf16, name="cb")
            cf = consts.tile([128, 2], f32, name="cf")
            heat = consts.tile([128, 128], bf16, name="heat")
            nc.vector.memset(heat, 0.0)
            # weights on the scalar queue (idle until the first exp);
            # sync + gpsimd queues carry every xt tile up front
            nc.scalar.dma_start(out=cb[:, 0:C_ID], in_=cb_d.ap()[:, 0:C_ID])
            nc.scalar.dma_start(out=cb[:, C_ID:], in_=cb_d.ap()[:, C_ID:])
            nc.scalar.dma_start(out=cf, in_=cf_d.ap())

            units = [(b, st) for st in range(NST) for b in range(BPC)]
            xt_sbs = []
            qeng = [nc.sync, nc.gpsimd]
            for i, (b, st) in enumerate(units):
                xt_sb = xtp.tile([128, NDB, ST], bf16, tag="xt",
                                 bufs=8, name=f"xt_sb{i}")
                src = xt_d.ap()[b, :, ST * st:ST * (st + 1)]
                src = src.rearrange("(db p) s -> p db s", p=128)
                nds = 4 if i == 0 else (2 if i == 1 else 1)
                step = NDB // nds
                for k in range(nds):
                    qeng[(2 * i + k) % 2].dma_start(
                        out=xt_sb[:, k * step:(k + 1) * step, :],
                        in_=src[:, k * step:(k + 1) * step, :])
                xt_sbs.append(xt_sb)

            wkq = cb[:, C_WKQ:C_WKQ + 1024].rearrange("p (db m) -> p db m",
                                                      db=NDB)
            wqv = cb[:, C_WQV:C_WQV + 1024].rearrange("p (db m) -> p db m",
                                                      db=NDB)
            ident = cb[:, C_ID:C_ID + 128]
            mask = cb[:, C_MASK:C_MASK + 128]
            ones_c = cb[:, C_ONES:C_ONES + NKB]

            # matmul heater: keeps HAM busy through the initial DMA fill
            heat_ps = ps_o.tile([128, 4, 128], f32, tag="o", name="heat_ps")
            for _ in range(N_HEAT):
                nc.tensor.matmul(heat_ps[:, 0, :], heat, heat,
                                 start=True, stop=True)

            kq_sbs, vn_sbs = [], []
            for b in range(BPC):
                kq_sb = per.tile([128, S], bf16, name=f"kq_sb{b}")
                vn_sb = per.tile([128, NKB, 65], bf16, name=f"vn_sb{b}")
                # gpsimd: emitted after its xt dma_starts, so the queue
                # issues all loads before blocking on the consts DMA
                nc.gpsimd.tensor_copy(
                    out=vn_sb[:, :, 64:65],
                    in_=ones_c.rearrange("p (t o) -> p t o", o=1))
                kq_sbs.append(kq_sb)
                vn_sbs.append(vn_sb)

            def filler_gen(i, b, st):
                """Projections and V-natural prep for unit (b, st).  The
                two passes run through a single-bank PSUM ring (frees a
                bank for ps_o triple-buffering)."""
                kq_sb, vn_sb = kq_sbs[b], vn_sbs[b]
                xt_sb = xt_sbs[i]
                p1 = pp.tile([128, 512], f32, tag="p12", name="p1")
                for db in range(NDB):
                    nc.tensor.matmul(p1, wkq[:, db, :], xt_sb[:, db, :],
                                     start=(db == 0), stop=(db == NDB - 1))
                    if db % 2 == 1:
                        yield None
                # PSUM->SBUF with bias add, bf16 out
                nc.vector.tensor_scalar_add(
                    out=kq_sb[0:64, ST * st:ST * (st + 1)],
                    in0=p1[0:64, :], scalar1=cf[0:64, 0:1])
                yield None
                p2 = pp.tile([128, 512], f32, tag="p12", name="p2")
                for db in range(NDB):
                    nc.tensor.matmul(p2, wqv[:, db, :], xt_sb[:, db, :],
                                     start=(db == 0), stop=(db == NDB - 1))
                    if db % 2 == 1:
                        yield None
                qv_sb = work.tile([128, ST], bf16, tag="qv", bufs=4,
                                  name="qv_sb")
                nc.vector.tensor_scalar_add(out=qv_sb, in0=p2,
                                            scalar1=cf[:, 1:2])
                yield None
                vn_ps = pp.tile([128, 4, 64], bf16, tag="p12", name="vn_ps")
                for c in range(4):
                    nc.tensor.transpose(
                        vn_ps[:, c, :],
                        qv_sb[64:128, 128 * c:128 * (c + 1)],
                        ident[64:128, 64:128])
                nc.vector.tensor_copy(
                    out=vn_sb[:, 4 * st:4 * st + 4, 0:64], in_=vn_ps)
                yield qv_sb

            def fill_chain():
                for i, (b, st) in enumerate(units):
                    for r in filler_gen(i, b, st):
                        if r is not None:
                            yield ("unit", i, b, st, r)
                        else:
                            yield ("step", i)

            chain = fill_chain()
            qv_units = {}
            exhausted = [False]

            def pull_until_unit(i):
                for kind, *rest in chain:
                    if kind == "unit":
                        qv_units[rest[0]] = rest[3]
                        if rest[0] >= i:
                            return
                exhausted[0] = True

            def pull_steps(n):
                got = 0
                while got < n:
                    try:
                        kind, *rest = next(chain)
                    except StopIteration:
                        exhausted[0] = True
                        return
                    if kind == "unit":
                        qv_units[rest[0]] = rest[3]
                    else:
                        got += 1

            def emit_o(st, kb, bbase, e_sb, o_ts, last):
                j0 = max(kb - 4 * st, 0)
                for b in range(BPC):
                    base = bbase * b
                    for qc in range(j0, 4):
                        nc.tensor.matmul(
                            o_ts[b][:, qc, 0:65],
                            e_sb[:, base + 128 * (qc - j0):
                                 base + 128 * (qc - j0) + 128],
                            vn_sbs[b][:, kb, :],
                            start=(kb == 0 and qc == j0),
                            stop=(last and qc == 3))

            EXP = mybir.ActivationFunctionType.Exp
            scale = float(HD) ** -0.5
            for st in range(NST):
                iB = 2 * st + 1
                if iB not in qv_units:
                    pull_until_unit(iB)
                qv_A = qv_units.pop(iB - 1)
                qv_B = qv_units.pop(iB)
                o_ts = [ps_o.tile([128, 4, 128], f32, tag="o",
                                  name=f"o_t{b}") for b in range(BPC)]
                n_att = 4 * st + 4
                pend = None
                for kb in range(n_att):
                    j = kb - 4 * st
                    w = ST if j < 0 else ST - 128 * j
                    qoff = 0 if j < 0 else 128 * j
                    # batch B's bank offset: diag j>=2 packs into bank 0
                    bbase = 512 if w > 256 else w
                    s_ps = ps_s.tile([128, 1024], f32, tag="s", name="s_ps")
                    nc.tensor.matmul(
                        s_ps[:, 0:w],
                        kq_sbs[0][0:64, 128 * kb:128 * kb + 128],
                        qv_A[0:64, qoff:qoff + w],
                        start=True, stop=(bbase == 512))
                    nc.tensor.matmul(
                        s_ps[:, bbase:bbase + w],
                        kq_sbs[1][0:64, 128 * kb:128 * kb + 128],
                        qv_B[0:64, qoff:qoff + w],
                        start=(bbase == 512), stop=True)
                    e_sb = work.tile([128, 1024], bf16, tag="e", bufs=4,
                                     name="e_sb")
                    if w == ST or bbase != 512:
                        nc.scalar.activation(out=e_sb[:, 0:bbase + w],
                                             in_=s_ps[:, 0:bbase + w],
                                             func=EXP, scale=scale)
                    else:
                        nc.scalar.activation(out=e_sb[:, 0:w],
                                             in_=s_ps[:, 0:w],
                                             func=EXP, scale=scale)
                        nc.scalar.activation(out=e_sb[:, 512:512 + w],
                                             in_=s_ps[:, 512:512 + w],
                                             func=EXP, scale=scale)
                    if j >= 0:
                        nc.vector.tensor_mul(out=e_sb[:, 0:128],
                                             in0=e_sb[:, 0:128], in1=mask)
                        nc.vector.tensor_mul(
                            out=e_sb[:, bbase:bbase + 128],
                            in0=e_sb[:, bbase:bbase + 128], in1=mask)
                    if pend is not None:
                        emit_o(st, *pend, o_ts, last=False)
                    pend = (kb, bbase, e_sb)
                    pull_steps(PULLS[st])
                    if exhausted[0]:
                        # tail heater: keep the PE dense (HAM warm)
                        # through the exp-paced last blocks; runs in the
                        # idle projection-PSUM ring
                        heat_t = pp.tile([128, 512], f32, tag="p12",
                                         name="heat_t")
                        for _ in range(2):
                            nc.tensor.matmul(heat_t[:, 0:128], heat, heat,
                                             start=True, stop=True)
                # final block: per batch, finish O then immediately drain
                # that batch's o bank (copy + DMA) so the next pair's
                # start=True never waits on it
                kb_l, bbase_l, e_l = pend
                j0 = max(kb_l - 4 * st, 0)
                for b in range(BPC):
                    base = bbase_l * b
                    for qc in range(j0, 4):
                        nc.tensor.matmul(
                            o_ts[b][:, qc, 0:65],
                            e_l[:, base + 128 * (qc - j0):
                                base + 128 * (qc - j0) + 128],
                            vn_sbs[b][:, kb_l, :],
                            start=False, stop=(qc == 3))
                    o_sb = work.tile([128, 4, 65], f32, tag="osb", bufs=4,
                                     name="o_sb")
                    # ScalarE: idle at pair ends, closer to PSUM, and keeps
                    # the copy off the DVE queue ahead of next-pair masks
                    nc.scalar.copy(out=o_sb, in_=o_ts[b][:, :, 0:65])
                    nc.sync.dma_start(out=out.ap()[b, st], in_=o_sb)

    nc.compile()
    return nc


def _pack_consts(Wq, bq, Wk, bk, Wv, bv):
    def packed_pair(wa, wb):
        pa = wa.reshape(NDB, 128, HD).transpose(1, 0, 2)
        pb = wb.reshape(NDB, 128, HD).transpose(1, 0, 2)
        return np.concatenate([pa, pb], axis=2).reshape(128, NDB * 128)

    cb = np.zeros((128, C_TOT), dtype=np.float32)
    cb[:, C_WKQ:C_WKQ + 1024] = packed_pair(Wk, Wq)
    cb[:, C_WQV:C_WQV + 1024] = packed_pair(Wq, Wv)
    cb[:, C_ID:C_ID + 128] = np.eye(128, dtype=np.float32)
    cb[:, C_MASK:C_MASK + 128] = (
        np.arange(128)[None, :] >= np.arange(128)[:, None])
    cb[:, C_ONES:] = 1.0
    cf = np.zeros((128, 2), dtype=np.float32)
    cf[:, 0] = np.concatenate([bk, bq])
    cf[:, 1] = np.concatenate([bq, bv])
    return np.ascontiguousarray(cb.astype(BF)), np.ascontiguousarray(cf)


def kernel(x, Wq, bq, Wk, bk, Wv, bv):
    if "nc" not in _cache:
        _cache["nc"] = _build()
    nc = _cache["nc"]

    x = np.asarray(x, dtype=np.float32).reshape(N_CORES, BPC, S, D)
    xt = np.ascontiguousarray(x.astype(BF).transpose(0, 1, 3, 2))
    cb, cf = _pack_consts(np.asarray(Wq, np.float32),
                          np.asarray(bq, np.float32),
                          np.asarray(Wk, np.float32),
                          np.asarray(bk, np.float32),
                          np.asarray(Wv, np.float32),
                          np.asarray(bv, np.float32))

    in_maps = []
    for c in range(N_CORES):
        in_maps.append({"xt": xt[c], "cb": cb, "cf": cf})

    res = bass_utils.run_bass_kernel_spmd(nc, in_maps,
                                          core_ids=list(range(N_CORES)),
                                          **_cache.get("run_kwargs", {}))
    _cache["last_result"] = res
    o = np.concatenate([res.results[c]["out"] for c in range(N_CORES)],
                       axis=0)                      # [B, NST, 128, 4, 65]
    o = o.transpose(0, 1, 3, 2, 4).reshape(B, S, 65)  # seq = st*512+t*128+p
    return np.ascontiguousarray(o[:, :, 0:64] / o[:, :, 64:65])
